# revision 2
# baseline (speedup 1.0000x reference)
"""Game-of-Life CNN (3x3 circular conv + double-heaviside) on 8 trn2 cores, v3.

Data-parallel over batch: 16 images -> 8 cores x 2. Per core:

  - Host pre-pads each image circularly -> fp8e4 [2050, 2052] rows
    (+94 zero rows after the last image so every tile loads 128 rows).
  - 17 row-tiles per image (126 output rows each), 2 half-tiles of 1024
    output columns -> 68 half-tiles per core.
  - conv: 4 DoubleRowSwInterleave fp8 matmuls per half-tile compute
    v = 0.25 * (2*neighbors + self) for even/odd pixel phases into one
    psum tile [128, 1024] = [even 512 | odd 512].  Weights are the
    0.25-scaled band matrices, SW-interleaved + column-reversed.
  - threshold (alive <=> conv count c in {5,6,7} <=> v in {1.25,1.5,1.75}):
      * S-assist tiles: ScalarE Square(v - 1.5) -> bf16, VectorE
        (q <= 0.15) -> u16 {0,1} (4x DVE mode).
      * V-solo tiles: custom-DVE ADD_RANGE_WRAP: u16(RNE(
          (v - 0.5625) + 4*[wrap once into +-1.25] )) -> {0,1}.
  - pack: 8 accumulating DoubleRow fp8e5 matmuls read the u16 booleans
    bitcast to fp8e5 (0x0001 -> denormal 2^-16) and pack 16 pixels per
    psum f32; x65536 copy -> u16 -> DMA (1/8 the output bytes).
  - host decodes the bit-packed output and casts to f32.
"""

import numpy as np
import ml_dtypes

import concourse.bass as bass
import concourse.bacc as bacc
import concourse.mybir as mybir
from concourse import tile
from concourse.bass_utils import run_bass_kernel_spmd

AP = bass.AP
F8 = mybir.dt.float8e4
F8E5 = mybir.dt.float8e5
F32 = mybir.dt.float32
BF16 = mybir.dt.bfloat16
U16 = mybir.dt.uint16
SWI = mybir.MatmulPerfMode.DoubleRowSwInterleave
DR = mybir.MatmulPerfMode.DoubleRow

NP_F8 = ml_dtypes.float8_e4m3
NP_F8E5 = ml_dtypes.float8_e5m2

B, H, W = 16, 2048, 2048
NCORES = 8
IPC = B // NCORES            # images per core
HP, WP = H + 2, W + 4        # padded rows; cols 2052 (2050 + 2 zero pad)
ROWS_OUT = 126
NT = 17                      # row-tiles per image (16*126 + 32)
NHT = IPC * NT * 2           # 68 half-tiles per core
TAILPAD = 128 - (HP - ROWS_OUT * (NT - 1))  # 94 extra rows after last image
NGRP = (NHT + 7) // 8        # pack groups of 8 half-tiles -> 9

GAMMA = 0.25


def _swi_layout(wA, wB):
    """Two [128,128] stationary mats -> [128,256] DoubleRowSwInterleave
    memory order (pairs interleaved per column, columns reversed)."""
    w = np.zeros((128, 2 * 128), np.float32)
    w[:, 0::2] = wA[:, ::-1]
    w[:, 1::2] = wB[:, ::-1]
    return w


def _weights_np():
    V2 = np.zeros((128, 128), np.float32)
    V1 = np.zeros((128, 128), np.float32)
    for o in range(128):
        for dp in range(3):
            p = o + dp
            if p <= 127:
                V2[p, o] = 2.0 * GAMMA
                V1[p, o] = (1.0 if dp == 1 else 2.0) * GAMMA
    Z = np.zeros((128, 128), np.float32)
    # even pixels j=2n need padded cols (2n:V2, 2n+1:V1) + (2n+2:V2)
    # odd  pixels j=2n+1 need (2n+1:V2) + (2n+2:V1, 2n+3:V2)
    wc = np.stack([_swi_layout(V2, V1), _swi_layout(V2, Z),
                   _swi_layout(Z, V2), _swi_layout(V1, V2)], axis=1)
    wp = np.zeros((128, 8, 2, 128), np.float32)
    for o in range(128):
        for k in range(8):
            wp[o, k, 0, o] = float(1 << (2 * k))
            wp[o, k, 1, o] = float(1 << (2 * k + 1))
    return wc.reshape(128, -1), wp.reshape(128, -1)


def _build_nc():
    nc = bacc.Bacc()
    NROWS = IPC * HP + TAILPAD
    x = nc.dram_tensor("x", [NROWS, WP], F8, kind="ExternalInput")
    wc_d = nc.dram_tensor("wc", [128, 4 * 256], F8, kind="ExternalInput")
    wp_d = nc.dram_tensor("wp", [128, 8 * 256], F8E5, kind="ExternalInput")
    y = nc.dram_tensor("y", [NGRP * 128, 512], U16, kind="ExternalOutput")

    with tile.TileContext(nc) as tc:
        with (
            tc.tile_pool(name="const", bufs=1) as cpool,
            tc.tile_pool(name="xp", bufs=3) as xpool,
            tc.tile_pool(name="q", bufs=3) as qpool,
            tc.tile_pool(name="t", bufs=4) as tpool,
            tc.tile_pool(name="ob", bufs=2) as opool,
            tc.tile_pool(name="cps", bufs=3, space="PSUM") as cps,
            tc.tile_pool(name="pps", bufs=2, space="PSUM") as pps,
        ):
            wc = cpool.tile([128, 4, 256], F8)
            nc.sync.dma_start(out=wc[:, :, :], in_=wc_d[:, :])
            wp = cpool.tile([128, 8, 2, 128], F8E5)
            nc.sync.dma_start(out=wp[:, :, :, :], in_=wp_d[:, :])
            biasq = cpool.tile([128, 1], F32)
            nc.vector.memset(biasq[:, :], -1.5)
            bias0 = cpool.tile([128, 1], F32)
            nc.vector.memset(bias0[:, :], 0.0)

            # batched input loads: per image, 4+4+4+4+1 row-tiles
            xp_tiles = {}  # (img, batch) -> (tile, count)
            batches = [(0, 4), (4, 4), (8, 4), (12, 4), (16, 1)]

            def load_batch(i, b0, cnt):
                xt = xpool.tile([128, 4, WP], F8, tag="xp")
                src0 = (i * HP + ROWS_OUT * b0)
                a0 = x[src0:src0 + 1, 0:1]
                src = AP(a0.tensor, a0.offset,
                         [[WP, 128], [ROWS_OUT * WP, cnt], [1, WP]])
                nc.sync.dma_start(out=xt[:, 0:cnt, :], in_=src)
                return xt

            grp_ps = None
            grp_ob = []
            for h in range(NHT):
                i, rem = divmod(h, NT * 2)
                t, s = divmod(rem, 2)
                bi = [bidx for bidx, (b0, cnt) in enumerate(batches)
                      if b0 <= t < b0 + cnt][0]
                b0, cnt = batches[bi]
                key = (i, bi)
                if key not in xp_tiles:
                    xp_tiles[key] = load_batch(i, b0, cnt)
                xt = xp_tiles[key]
                tb = t - b0

                # ---- conv: 4 SwInterleave matmuls -> psum [even|odd] ----
                ps = cps.tile([128, 1024], F32, tag="cps")
                base = tb * WP + s * 1024

                def prhs(off):
                    a0 = xt[:, 0:1, 0:1]
                    return AP(a0.tensor, a0.offset + base + off,
                              [list(a0.ap[0]), [1, 2], [2, 512]])

                nc.tensor.matmul(ps[:, 0:512], lhsT=wc[:, 0, :], rhs=prhs(0),
                                 start=True, stop=False, perf_mode=SWI)
                nc.tensor.matmul(ps[:, 0:512], lhsT=wc[:, 1, :], rhs=prhs(2),
                                 start=False, stop=True, perf_mode=SWI)
                nc.tensor.matmul(ps[:, 512:1024], lhsT=wc[:, 2, :], rhs=prhs(0),
                                 start=True, stop=False, perf_mode=SWI)
                nc.tensor.matmul(ps[:, 512:1024], lhsT=wc[:, 3, :], rhs=prhs(2),
                                 start=False, stop=True, perf_mode=SWI)

                # ---- threshold -> u16 {0,1} ----
                tt = tpool.tile([128, 1024], U16, tag="t")
                if h % 4 != 0:
                    q = qpool.tile([128, 1024], BF16, tag="q")
                    nc.scalar.activation(q[:, :], ps[:, :],
                                         mybir.ActivationFunctionType.Square,
                                         bias=biasq[:, 0:1], scale=1.0)
                    nc.vector.tensor_scalar(tt[:, :], q[:, :], 0.15, None,
                                            mybir.AluOpType.is_le)
                else:
                    nc.vector.add_range_wrap(tt[:, :], ps[:, :],
                                             -0.5625, 1.25, 4.0)

                # ---- pack: 8 DoubleRow matmuls -> pack psum cols ----
                g, m = divmod(h, 8)
                if m == 0:
                    grp_ps = pps.tile([128, 512], F32, tag="pps")
                te5 = tt[:, 0:1].bitcast(F8E5)
                for k in range(8):
                    rhs_p = AP(te5.tensor, te5.offset + 2 * 64 * k,
                               [list(te5.ap[0]), [1024, 2], [2, 64]])
                    nc.tensor.matmul(grp_ps[:, 64 * m:64 * m + 64],
                                     lhsT=wp[:, k, :, :], rhs=rhs_p,
                                     start=(k == 0), stop=(k == 7),
                                     perf_mode=DR)

                if m == 7 or h == NHT - 1:
                    ncols = 64 * (m + 1)
                    ob = opool.tile([128, 512], U16, tag="ob")
                    nc.vector.tensor_scalar(ob[:, 0:ncols],
                                            grp_ps[:, 0:ncols],
                                            65536.0, None,
                                            mybir.AluOpType.mult)
                    nc.sync.dma_start(
                        out=y[128 * g:128 * g + 128, 0:ncols],
                        in_=ob[:, 0:ncols])
    nc.finalize()
    return nc


def _pad_circular(xb):
    """[IPC, H, W] f32 -> fp8 padded [IPC*HP + TAILPAD, WP]."""
    out = np.zeros((IPC * HP + TAILPAD, WP), NP_F8)
    x8 = xb.astype(NP_F8)
    for i in range(IPC):
        r0 = i * HP
        out[r0 + 1:r0 + H + 1, 1:W + 1] = x8[i]
        out[r0, 1:W + 1] = x8[i, H - 1]
        out[r0 + H + 1, 1:W + 1] = x8[i, 0]
        out[r0:r0 + HP, 0] = out[r0:r0 + HP, W]
        out[r0:r0 + HP, W + 1] = out[r0:r0 + HP, 1]
    return out


def _decode(yc):
    """[NGRP*128, 512] u16 -> [IPC, H, W] u8 booleans."""
    Y = yc.reshape(NGRP, 128, 8, 64)[:, :, :, :]        # g, p, m, j
    Y = Y.transpose(0, 2, 1, 3).reshape(NGRP * 8, 128, 64)[:NHT]  # h, p, j
    bits = (Y[:, :, :, None] >> np.arange(16, dtype=np.uint16)) & 1
    # bit b = 2k+e -> pixel 2*(64k + j) + e ; [h, p, j, k, e] -> [h, p, k, j, e]
    bits = bits.reshape(NHT, 128, 64, 8, 2).transpose(0, 1, 3, 2, 4)
    blk = bits.reshape(NHT, 128, 1024).astype(np.uint8)
    out = np.empty((IPC, H, W), np.uint8)
    for h in range(NHT):
        i, rem = divmod(h, NT * 2)
        t, s = divmod(rem, 2)
        nrows = ROWS_OUT if t < NT - 1 else H - ROWS_OUT * (NT - 1)
        out[i, ROWS_OUT * t:ROWS_OUT * t + nrows, 1024 * s:1024 * s + 1024] = \
            blk[h, :nrows, :]
    return out


def _run(inputs, **kw):
    x = np.asarray(inputs["x"], dtype=np.float32).reshape(B, H, W)
    bias = float(np.asarray(inputs.get("bias", np.zeros(1))).reshape(-1)[0])
    # alive <=> count c in [4.5-bias, 7.5-bias]; for |bias|<0.5 that is {5,6,7}
    lo = int(np.ceil(4.5 - bias - 1e-9))
    hi = int(np.ceil(7.5 - bias - 1e-9)) - 1
    assert (lo, hi) == (5, 7), f"unexpected threshold band {(lo, hi)}"

    wc, wp = _weights_np()
    nc = _build_nc()
    in_maps = []
    for c in range(NCORES):
        xb = _pad_circular(x[c * IPC:(c + 1) * IPC])
        in_maps.append({"x": xb, "wc": wc.astype(NP_F8),
                        "wp": wp.astype(NP_F8E5)})
    res = run_bass_kernel_spmd(nc, in_maps, core_ids=list(range(NCORES)), **kw)
    out = np.empty((B, 1, H, W), np.float32)
    for c in range(NCORES):
        out[c * IPC:(c + 1) * IPC, 0] = _decode(res.results[c]["y"])
    return out, res


def kernel(**inputs) -> np.ndarray:
    out, _ = _run(inputs)
    return out


# revision 3
# speedup vs baseline: 1.0635x; 1.0635x over previous
"""Game-of-Life CNN (3x3 circular conv + double-heaviside) on 8 trn2 cores, v3.

Data-parallel over batch: 16 images -> 8 cores x 2. Per core:

  - Host pre-pads each image circularly -> fp8e4 [2050, 2052] rows
    (+94 zero rows after the last image so every tile loads 128 rows).
  - 17 row-tiles per image (126 output rows each), 2 half-tiles of 1024
    output columns -> 68 half-tiles per core.
  - conv: 4 DoubleRowSwInterleave fp8 matmuls per half-tile compute
    v = 0.25 * (2*neighbors + self) for even/odd pixel phases into one
    psum tile [128, 1024] = [even 512 | odd 512].  Weights are the
    0.25-scaled band matrices, SW-interleaved + column-reversed.
  - threshold (alive <=> conv count c in {5,6,7} <=> v in {1.25,1.5,1.75}):
      * S-assist tiles: ScalarE Square(v - 1.5) -> bf16, VectorE
        (q <= 0.15) -> u16 {0,1} (4x DVE mode).
      * V-solo tiles: custom-DVE ADD_RANGE_WRAP: u16(RNE(
          (v - 0.5625) + 4*[wrap once into +-1.25] )) -> {0,1}.
  - pack: 8 accumulating DoubleRow fp8e5 matmuls read the u16 booleans
    bitcast to fp8e5 (0x0001 -> denormal 2^-16) and pack 16 pixels per
    psum f32; x65536 copy -> u16 -> DMA (1/8 the output bytes).
  - host decodes the bit-packed output and casts to f32.
"""

import numpy as np
import ml_dtypes

import concourse.bass as bass
import concourse.bacc as bacc
import concourse.mybir as mybir
from concourse import tile
from concourse.bass_utils import run_bass_kernel_spmd

AP = bass.AP
F8 = mybir.dt.float8e4
F8E5 = mybir.dt.float8e5
F32 = mybir.dt.float32
BF16 = mybir.dt.bfloat16
U16 = mybir.dt.uint16
SWI = mybir.MatmulPerfMode.DoubleRowSwInterleave
DR = mybir.MatmulPerfMode.DoubleRow

NP_F8 = ml_dtypes.float8_e4m3
NP_F8E5 = ml_dtypes.float8_e5m2

B, H, W = 16, 2048, 2048
NCORES = 8
IPC = B // NCORES            # images per core
HP, WP = H + 2, W + 4        # padded rows; cols 2052 (2050 + 2 zero pad)
ROWS_OUT = 126
NT = 17                      # row-tiles per image (16*126 + 32)
NHT = IPC * NT * 2           # 68 half-tiles per core
TAILPAD = 128 - (HP - ROWS_OUT * (NT - 1))  # 94 extra rows after last image
NGRP = (NHT + 7) // 8        # pack groups of 8 half-tiles -> 9

GAMMA = 0.25


def _swi_layout(wA, wB):
    """Two [128,128] stationary mats -> [128,256] DoubleRowSwInterleave
    memory order (pairs interleaved per column, columns reversed)."""
    w = np.zeros((128, 2 * 128), np.float32)
    w[:, 0::2] = wA[:, ::-1]
    w[:, 1::2] = wB[:, ::-1]
    return w


def _weights_np():
    V2 = np.zeros((128, 128), np.float32)
    V1 = np.zeros((128, 128), np.float32)
    for o in range(128):
        for dp in range(3):
            p = o + dp
            if p <= 127:
                V2[p, o] = 2.0 * GAMMA
                V1[p, o] = (1.0 if dp == 1 else 2.0) * GAMMA
    Z = np.zeros((128, 128), np.float32)
    # even pixels j=2n need padded cols (2n:V2, 2n+1:V1) + (2n+2:V2)
    # odd  pixels j=2n+1 need (2n+1:V2) + (2n+2:V1, 2n+3:V2)
    wc = np.stack([_swi_layout(V2, V1), _swi_layout(V2, Z),
                   _swi_layout(Z, V2), _swi_layout(V1, V2)], axis=1)
    wp = np.zeros((128, 8, 2, 128), np.float32)
    for o in range(128):
        for k in range(8):
            wp[o, k, 0, o] = float(1 << (2 * k))
            wp[o, k, 1, o] = float(1 << (2 * k + 1))
    return wc.reshape(128, -1), wp.reshape(128, -1)


def _build_nc():
    nc = bacc.Bacc()
    NROWS = IPC * HP + TAILPAD
    x = nc.dram_tensor("x", [NROWS, WP], F8, kind="ExternalInput")
    wc_d = nc.dram_tensor("wc", [128, 4 * 256], F8, kind="ExternalInput")
    wp_d = nc.dram_tensor("wp", [128, 8 * 256], F8E5, kind="ExternalInput")
    y = nc.dram_tensor("y", [NGRP * 128, 512], U16, kind="ExternalOutput")

    with tile.TileContext(nc) as tc:
        with (
            tc.tile_pool(name="const", bufs=1) as cpool,
            tc.tile_pool(name="xp", bufs=4) as xpool,
            tc.tile_pool(name="q", bufs=3) as qpool,
            tc.tile_pool(name="t", bufs=4) as tpool,
            tc.tile_pool(name="ob", bufs=2) as opool,
            tc.tile_pool(name="cps", bufs=3, space="PSUM") as cps,
            tc.tile_pool(name="pps", bufs=2, space="PSUM") as pps,
        ):
            wc = cpool.tile([128, 4, 256], F8)
            nc.sync.dma_start(out=wc[:, :, :], in_=wc_d[:, :])
            wp = cpool.tile([128, 8, 2, 128], F8E5)
            nc.sync.dma_start(out=wp[:, :, :, :], in_=wp_d[:, :])
            biasq = cpool.tile([128, 1], F32)
            nc.vector.memset(biasq[:, :], -1.5)
            bias0 = cpool.tile([128, 1], F32)
            nc.vector.memset(bias0[:, :], 0.0)

            # batched input loads: per image, 4+4+4+4+1 row-tiles
            xp_tiles = {}  # (img, batch) -> (tile, count)
            batches = [(0, 1), (1, 2), (3, 4), (7, 4), (11, 4), (15, 2)]

            def load_batch(i, b0, cnt):
                xt = xpool.tile([128, 4, WP], F8, tag="xp")
                src0 = (i * HP + ROWS_OUT * b0)
                a0 = x[src0:src0 + 1, 0:1]
                src = AP(a0.tensor, a0.offset,
                         [[WP, 128], [ROWS_OUT * WP, cnt], [1, WP]])
                nc.sync.dma_start(out=xt[:, 0:cnt, :], in_=src)
                return xt

            grp_ps = None
            grp_ob = []
            for h in range(NHT):
                i, rem = divmod(h, NT * 2)
                t, s = divmod(rem, 2)
                bi = [bidx for bidx, (b0, cnt) in enumerate(batches)
                      if b0 <= t < b0 + cnt][0]
                b0, cnt = batches[bi]
                key = (i, bi)
                if key not in xp_tiles:
                    xp_tiles[key] = load_batch(i, b0, cnt)
                xt = xp_tiles[key]
                tb = t - b0

                # ---- conv: 4 SwInterleave matmuls -> psum [even|odd] ----
                ps = cps.tile([128, 1024], F32, tag="cps")
                base = tb * WP + s * 1024

                def prhs(off):
                    a0 = xt[:, 0:1, 0:1]
                    return AP(a0.tensor, a0.offset + base + off,
                              [list(a0.ap[0]), [1, 2], [2, 512]])

                nc.tensor.matmul(ps[:, 0:512], lhsT=wc[:, 0, :], rhs=prhs(0),
                                 start=True, stop=False, perf_mode=SWI)
                nc.tensor.matmul(ps[:, 0:512], lhsT=wc[:, 1, :], rhs=prhs(2),
                                 start=False, stop=True, perf_mode=SWI)
                nc.tensor.matmul(ps[:, 512:1024], lhsT=wc[:, 2, :], rhs=prhs(0),
                                 start=True, stop=False, perf_mode=SWI)
                nc.tensor.matmul(ps[:, 512:1024], lhsT=wc[:, 3, :], rhs=prhs(2),
                                 start=False, stop=True, perf_mode=SWI)

                # ---- threshold -> u16 {0,1} ----
                tt = tpool.tile([128, 1024], U16, tag="t")
                if h % 4 != 0:
                    q = qpool.tile([128, 1024], BF16, tag="q")
                    nc.scalar.activation(q[:, :], ps[:, :],
                                         mybir.ActivationFunctionType.Square,
                                         bias=biasq[:, 0:1], scale=1.0)
                    nc.vector.tensor_scalar(tt[:, :], q[:, :], 0.15, None,
                                            mybir.AluOpType.is_le)
                else:
                    nc.vector.add_range_wrap(tt[:, :], ps[:, :],
                                             -0.5625, 1.25, 4.0)

                # ---- pack: 8 DoubleRow matmuls -> pack psum cols ----
                g, m = divmod(h, 8)
                if m == 0:
                    grp_ps = pps.tile([128, 512], F32, tag="pps")
                te5 = tt[:, 0:1].bitcast(F8E5)
                for k in range(8):
                    rhs_p = AP(te5.tensor, te5.offset + 2 * 64 * k,
                               [list(te5.ap[0]), [1024, 2], [2, 64]])
                    nc.tensor.matmul(grp_ps[:, 64 * m:64 * m + 64],
                                     lhsT=wp[:, k, :, :], rhs=rhs_p,
                                     start=(k == 0), stop=(k == 7),
                                     perf_mode=DR)

                if m == 7 or h == NHT - 1:
                    ncols = 64 * (m + 1)
                    ob = opool.tile([128, 512], U16, tag="ob")
                    nc.vector.tensor_scalar(ob[:, 0:ncols],
                                            grp_ps[:, 0:ncols],
                                            65536.0, None,
                                            mybir.AluOpType.mult)
                    nc.sync.dma_start(
                        out=y[128 * g:128 * g + 128, 0:ncols],
                        in_=ob[:, 0:ncols])
    nc.finalize()
    return nc


def _pad_circular(xb):
    """[IPC, H, W] f32 -> fp8 padded [IPC*HP + TAILPAD, WP]."""
    out = np.zeros((IPC * HP + TAILPAD, WP), NP_F8)
    x8 = xb.astype(NP_F8)
    for i in range(IPC):
        r0 = i * HP
        out[r0 + 1:r0 + H + 1, 1:W + 1] = x8[i]
        out[r0, 1:W + 1] = x8[i, H - 1]
        out[r0 + H + 1, 1:W + 1] = x8[i, 0]
        out[r0:r0 + HP, 0] = out[r0:r0 + HP, W]
        out[r0:r0 + HP, W + 1] = out[r0:r0 + HP, 1]
    return out


def _decode(yc):
    """[NGRP*128, 512] u16 -> [IPC, H, W] u8 booleans."""
    Y = yc.reshape(NGRP, 128, 8, 64)[:, :, :, :]        # g, p, m, j
    Y = Y.transpose(0, 2, 1, 3).reshape(NGRP * 8, 128, 64)[:NHT]  # h, p, j
    bits = (Y[:, :, :, None] >> np.arange(16, dtype=np.uint16)) & 1
    # bit b = 2k+e -> pixel 2*(64k + j) + e ; [h, p, j, k, e] -> [h, p, k, j, e]
    bits = bits.reshape(NHT, 128, 64, 8, 2).transpose(0, 1, 3, 2, 4)
    blk = bits.reshape(NHT, 128, 1024).astype(np.uint8)
    out = np.empty((IPC, H, W), np.uint8)
    for h in range(NHT):
        i, rem = divmod(h, NT * 2)
        t, s = divmod(rem, 2)
        nrows = ROWS_OUT if t < NT - 1 else H - ROWS_OUT * (NT - 1)
        out[i, ROWS_OUT * t:ROWS_OUT * t + nrows, 1024 * s:1024 * s + 1024] = \
            blk[h, :nrows, :]
    return out


def _run(inputs, **kw):
    x = np.asarray(inputs["x"], dtype=np.float32).reshape(B, H, W)
    bias = float(np.asarray(inputs.get("bias", np.zeros(1))).reshape(-1)[0])
    # alive <=> count c in [4.5-bias, 7.5-bias]; for |bias|<0.5 that is {5,6,7}
    lo = int(np.ceil(4.5 - bias - 1e-9))
    hi = int(np.ceil(7.5 - bias - 1e-9)) - 1
    assert (lo, hi) == (5, 7), f"unexpected threshold band {(lo, hi)}"

    wc, wp = _weights_np()
    nc = _build_nc()
    in_maps = []
    for c in range(NCORES):
        xb = _pad_circular(x[c * IPC:(c + 1) * IPC])
        in_maps.append({"x": xb, "wc": wc.astype(NP_F8),
                        "wp": wp.astype(NP_F8E5)})
    res = run_bass_kernel_spmd(nc, in_maps, core_ids=list(range(NCORES)), **kw)
    out = np.empty((B, 1, H, W), np.float32)
    for c in range(NCORES):
        out[c * IPC:(c + 1) * IPC, 0] = _decode(res.results[c]["y"])
    return out, res


def kernel(**inputs) -> np.ndarray:
    out, _ = _run(inputs)
    return out


# revision 4
# speedup vs baseline: 1.0663x; 1.0026x over previous
"""Game-of-Life CNN (3x3 circular conv + double-heaviside) on 8 trn2 cores, v3.

Data-parallel over batch: 16 images -> 8 cores x 2. Per core:

  - Host pre-pads each image circularly -> fp8e4 [2050, 2052] rows
    (+94 zero rows after the last image so every tile loads 128 rows).
  - 17 row-tiles per image (126 output rows each), 2 half-tiles of 1024
    output columns -> 68 half-tiles per core.
  - conv: 4 DoubleRowSwInterleave fp8 matmuls per half-tile compute
    v = 0.25 * (2*neighbors + self) for even/odd pixel phases into one
    psum tile [128, 1024] = [even 512 | odd 512].  Weights are the
    0.25-scaled band matrices, SW-interleaved + column-reversed.
  - threshold (alive <=> conv count c in {5,6,7} <=> v in {1.25,1.5,1.75}):
      * S-assist tiles: ScalarE Square(v - 1.5) -> bf16, VectorE
        (q <= 0.15) -> u16 {0,1} (4x DVE mode).
      * V-solo tiles: custom-DVE ADD_RANGE_WRAP: u16(RNE(
          (v - 0.5625) + 4*[wrap once into +-1.25] )) -> {0,1}.
  - pack: 8 accumulating DoubleRow fp8e5 matmuls read the u16 booleans
    bitcast to fp8e5 (0x0001 -> denormal 2^-16) and pack 16 pixels per
    psum f32; x65536 copy -> u16 -> DMA (1/8 the output bytes).
  - host decodes the bit-packed output and casts to f32.
"""

import numpy as np
import ml_dtypes

import concourse.bass as bass
import concourse.bacc as bacc
import concourse.mybir as mybir
from concourse import tile
from concourse.bass_utils import run_bass_kernel_spmd

AP = bass.AP
F8 = mybir.dt.float8e4
F8E5 = mybir.dt.float8e5
F32 = mybir.dt.float32
BF16 = mybir.dt.bfloat16
U16 = mybir.dt.uint16
SWI = mybir.MatmulPerfMode.DoubleRowSwInterleave
DR = mybir.MatmulPerfMode.DoubleRow

NP_F8 = ml_dtypes.float8_e4m3
NP_F8E5 = ml_dtypes.float8_e5m2

B, H, W = 16, 2048, 2048
NCORES = 8
IPC = B // NCORES            # images per core
HP, WP = H + 2, W + 4        # padded rows; cols 2052 (2050 + 2 zero pad)
ROWS_OUT = 126
NT = 17                      # row-tiles per image (16*126 + 32)
NHT = IPC * NT * 2           # 68 half-tiles per core
TAILPAD = 128 - (HP - ROWS_OUT * (NT - 1))  # 94 extra rows after last image
NGRP = (NHT + 7) // 8        # pack groups of 8 half-tiles -> 9

GAMMA = 0.25


def _swi_layout(wA, wB):
    """Two [128,128] stationary mats -> [128,256] DoubleRowSwInterleave
    memory order (pairs interleaved per column, columns reversed)."""
    w = np.zeros((128, 2 * 128), np.float32)
    w[:, 0::2] = wA[:, ::-1]
    w[:, 1::2] = wB[:, ::-1]
    return w


def _weights_np():
    V2 = np.zeros((128, 128), np.float32)
    V1 = np.zeros((128, 128), np.float32)
    for o in range(128):
        for dp in range(3):
            p = o + dp
            if p <= 127:
                V2[p, o] = 2.0 * GAMMA
                V1[p, o] = (1.0 if dp == 1 else 2.0) * GAMMA
    Z = np.zeros((128, 128), np.float32)
    # even pixels j=2n need padded cols (2n:V2, 2n+1:V1) + (2n+2:V2)
    # odd  pixels j=2n+1 need (2n+1:V2) + (2n+2:V1, 2n+3:V2)
    wc = np.stack([_swi_layout(V2, V1), _swi_layout(V2, Z),
                   _swi_layout(Z, V2), _swi_layout(V1, V2)], axis=1)
    wp = np.zeros((128, 8, 2, 128), np.float32)
    for o in range(128):
        for k in range(8):
            wp[o, k, 0, o] = float(1 << (2 * k))
            wp[o, k, 1, o] = float(1 << (2 * k + 1))
    return wc.reshape(128, -1), wp.reshape(128, -1)


def _build_nc():
    nc = bacc.Bacc()
    NROWS = IPC * HP + TAILPAD
    x = nc.dram_tensor("x", [NROWS, WP], F8, kind="ExternalInput")
    wc_d = nc.dram_tensor("wc", [128, 4 * 256], F8, kind="ExternalInput")
    wp_d = nc.dram_tensor("wp", [128, 8 * 256], F8E5, kind="ExternalInput")
    y = nc.dram_tensor("y", [NGRP * 128, 512], U16, kind="ExternalOutput")

    with tile.TileContext(nc) as tc:
        with (
            tc.tile_pool(name="const", bufs=1) as cpool,
            tc.tile_pool(name="xp", bufs=4) as xpool,
            tc.tile_pool(name="q", bufs=3) as qpool,
            tc.tile_pool(name="t", bufs=4) as tpool,
            tc.tile_pool(name="ob", bufs=2) as opool,
            tc.tile_pool(name="cps", bufs=3, space="PSUM") as cps,
            tc.tile_pool(name="pps", bufs=2, space="PSUM") as pps,
        ):
            wc = cpool.tile([128, 4, 256], F8)
            nc.sync.dma_start(out=wc[:, :, :], in_=wc_d[:, :])
            wp = cpool.tile([128, 8, 2, 128], F8E5)
            nc.sync.dma_start(out=wp[:, :, :, :], in_=wp_d[:, :])
            biasq = cpool.tile([128, 1], F32)
            nc.vector.memset(biasq[:, :], -1.5)
            bias0 = cpool.tile([128, 1], F32)
            nc.vector.memset(bias0[:, :], 0.0)

            # batched input loads: per image, 4+4+4+4+1 row-tiles
            xp_tiles = {}  # (img, batch) -> (tile, count)
            batches = [(0, 1), (1, 2), (3, 4), (7, 4), (11, 4), (15, 2)]

            def load_batch(i, b0, cnt):
                xt = xpool.tile([128, 4, WP], F8, tag="xp")
                src0 = (i * HP + ROWS_OUT * b0)
                a0 = x[src0:src0 + 1, 0:1]
                src = AP(a0.tensor, a0.offset,
                         [[WP, 128], [ROWS_OUT * WP, cnt], [1, WP]])
                nc.sync.dma_start(out=xt[:, 0:cnt, :], in_=src)
                return xt

            grp_ps = None
            grp_ob = []
            for h in range(NHT):
                i, rem = divmod(h, NT * 2)
                t, s = divmod(rem, 2)
                bi = [bidx for bidx, (b0, cnt) in enumerate(batches)
                      if b0 <= t < b0 + cnt][0]
                b0, cnt = batches[bi]
                key = (i, bi)
                if key not in xp_tiles:
                    xp_tiles[key] = load_batch(i, b0, cnt)
                xt = xp_tiles[key]
                tb = t - b0

                # ---- conv: 4 SwInterleave matmuls -> psum [even|odd] ----
                ps = cps.tile([128, 1024], F32, tag="cps")
                base = tb * WP + s * 1024

                def prhs(off):
                    a0 = xt[:, 0:1, 0:1]
                    return AP(a0.tensor, a0.offset + base + off,
                              [list(a0.ap[0]), [1, 2], [2, 512]])

                nc.tensor.matmul(ps[:, 0:512], lhsT=wc[:, 0, :], rhs=prhs(0),
                                 start=True, stop=False, perf_mode=SWI)
                nc.tensor.matmul(ps[:, 0:512], lhsT=wc[:, 1, :], rhs=prhs(2),
                                 start=False, stop=True, perf_mode=SWI)
                nc.tensor.matmul(ps[:, 512:1024], lhsT=wc[:, 2, :], rhs=prhs(0),
                                 start=True, stop=False, perf_mode=SWI)
                nc.tensor.matmul(ps[:, 512:1024], lhsT=wc[:, 3, :], rhs=prhs(2),
                                 start=False, stop=True, perf_mode=SWI)

                # ---- threshold -> u16 {0,1} ----
                tt = tpool.tile([128, 1024], U16, tag="t")
                if h % 4 != 2:
                    q = qpool.tile([128, 1024], BF16, tag="q")
                    nc.scalar.activation(q[:, :], ps[:, :],
                                         mybir.ActivationFunctionType.Square,
                                         bias=biasq[:, 0:1], scale=1.0)
                    nc.vector.tensor_scalar(tt[:, :], q[:, :], 0.15, None,
                                            mybir.AluOpType.is_le)
                else:
                    nc.vector.add_range_wrap(tt[:, :], ps[:, :],
                                             -0.5625, 1.25, 4.0)

                # ---- pack: 8 DoubleRow matmuls -> pack psum cols ----
                g, m = divmod(h, 8)
                if m == 0:
                    grp_ps = pps.tile([128, 512], F32, tag="pps")
                te5 = tt[:, 0:1].bitcast(F8E5)
                for k in range(8):
                    rhs_p = AP(te5.tensor, te5.offset + 2 * 64 * k,
                               [list(te5.ap[0]), [1024, 2], [2, 64]])
                    nc.tensor.matmul(grp_ps[:, 64 * m:64 * m + 64],
                                     lhsT=wp[:, k, :, :], rhs=rhs_p,
                                     start=(k == 0), stop=(k == 7),
                                     perf_mode=DR)

                if m == 7 or h == NHT - 1:
                    ncols = 64 * (m + 1)
                    ob = opool.tile([128, 512], U16, tag="ob")
                    nc.vector.tensor_scalar(ob[:, 0:ncols],
                                            grp_ps[:, 0:ncols],
                                            65536.0, None,
                                            mybir.AluOpType.mult)
                    nc.sync.dma_start(
                        out=y[128 * g:128 * g + 128, 0:ncols],
                        in_=ob[:, 0:ncols])
    nc.finalize()
    return nc


def _pad_circular(xb):
    """[IPC, H, W] f32 -> fp8 padded [IPC*HP + TAILPAD, WP]."""
    out = np.zeros((IPC * HP + TAILPAD, WP), NP_F8)
    x8 = xb.astype(NP_F8)
    for i in range(IPC):
        r0 = i * HP
        out[r0 + 1:r0 + H + 1, 1:W + 1] = x8[i]
        out[r0, 1:W + 1] = x8[i, H - 1]
        out[r0 + H + 1, 1:W + 1] = x8[i, 0]
        out[r0:r0 + HP, 0] = out[r0:r0 + HP, W]
        out[r0:r0 + HP, W + 1] = out[r0:r0 + HP, 1]
    return out


def _decode(yc):
    """[NGRP*128, 512] u16 -> [IPC, H, W] u8 booleans."""
    Y = yc.reshape(NGRP, 128, 8, 64)[:, :, :, :]        # g, p, m, j
    Y = Y.transpose(0, 2, 1, 3).reshape(NGRP * 8, 128, 64)[:NHT]  # h, p, j
    bits = (Y[:, :, :, None] >> np.arange(16, dtype=np.uint16)) & 1
    # bit b = 2k+e -> pixel 2*(64k + j) + e ; [h, p, j, k, e] -> [h, p, k, j, e]
    bits = bits.reshape(NHT, 128, 64, 8, 2).transpose(0, 1, 3, 2, 4)
    blk = bits.reshape(NHT, 128, 1024).astype(np.uint8)
    out = np.empty((IPC, H, W), np.uint8)
    for h in range(NHT):
        i, rem = divmod(h, NT * 2)
        t, s = divmod(rem, 2)
        nrows = ROWS_OUT if t < NT - 1 else H - ROWS_OUT * (NT - 1)
        out[i, ROWS_OUT * t:ROWS_OUT * t + nrows, 1024 * s:1024 * s + 1024] = \
            blk[h, :nrows, :]
    return out


def _run(inputs, **kw):
    x = np.asarray(inputs["x"], dtype=np.float32).reshape(B, H, W)
    bias = float(np.asarray(inputs.get("bias", np.zeros(1))).reshape(-1)[0])
    # alive <=> count c in [4.5-bias, 7.5-bias]; for |bias|<0.5 that is {5,6,7}
    lo = int(np.ceil(4.5 - bias - 1e-9))
    hi = int(np.ceil(7.5 - bias - 1e-9)) - 1
    assert (lo, hi) == (5, 7), f"unexpected threshold band {(lo, hi)}"

    wc, wp = _weights_np()
    nc = _build_nc()
    in_maps = []
    for c in range(NCORES):
        xb = _pad_circular(x[c * IPC:(c + 1) * IPC])
        in_maps.append({"x": xb, "wc": wc.astype(NP_F8),
                        "wp": wp.astype(NP_F8E5)})
    res = run_bass_kernel_spmd(nc, in_maps, core_ids=list(range(NCORES)), **kw)
    out = np.empty((B, 1, H, W), np.float32)
    for c in range(NCORES):
        out[c * IPC:(c + 1) * IPC, 0] = _decode(res.results[c]["y"])
    return out, res


def kernel(**inputs) -> np.ndarray:
    out, _ = _run(inputs)
    return out
